# revision 27
# baseline (speedup 1.0000x reference)
"""Trainium2 Bass kernel for the RGB-D cross-attention gate module.

Math shortcut: the module returns only gate = sigmoid(bn3(mlp2(relu(bn2(mlp1(gap))))))
where gap = spatial mean of (att_r + att_b + rgb + dep1).  Summing att_r over
spatial j gives proj_rgb @ s with s[i] = sum_j attn[i, j], so the full N x N
attention never needs materializing - only softmax column denominators d[j]
and the attention row-sum vector s.

Sharding: 8 cores = 4 samples x 2 column-halves of the attention (softmax is
over rows i, so a column shard is fully local; host adds the two 64-vector
partials per sample and applies the tiny MLP during gather).

Per-core pipeline (matmuls bf16, fp32 PSUM accumulate):
  conv1x1+BN+ReLU -> dep1; projections prgb/pdep; E^T tiles (j on partitions,
  i free) -> exp on ACT at (128,2048) grain with accum_out giving d[j] for
  free; P stored in SBUF as fp8e5 (reusing the dep-input slots via tag
  sharing); s-pass contracts P against 4096/d (replicated fp8e5 stationary,
  DoubleRow-paired j-tiles => 0.5 cyc/row) into 64-row-replicated psum chunks
  that double as the partition broadcast; one fused DVE multiply-accumulate
  against prs=(prgb+pdep) with scale 1/4096 plus the rsum terms yields the
  64-vector partial output.
"""

import numpy as np
import ml_dtypes

import concourse.bass as bass
import concourse.bacc as bacc
import concourse.mybir as mybir
import concourse.tile as tile
from concourse.bass_utils import run_bass_kernel_spmd
EPS = 1e-5
N = 4096          # spatial positions (64 x 64)
NH = 2048         # this core's attention-column half
C = 64            # channels
C1 = 256          # dep input channels
BF16 = mybir.dt.bfloat16
FP8 = mybir.dt.float8e5
F32 = mybir.dt.float32
NPBF16 = ml_dtypes.bfloat16


def build_core_program(stage="full"):
    # Bacc (not plain Bass): its compile() pass legalizes multi-sem waits into
    # sequencer instructions; walrus rejects >1 sync wait per compute instr.
    nc = bacc.Bacc("TRN2", target_bir_lowering=False)

    # ---- DRAM I/O (per-core shards; all cores run this same program) ----
    rgb_d = nc.dram_tensor("rgb", (C, N), BF16, kind="ExternalInput")
    depA_d = nc.dram_tensor("depA", (128, N), BF16, kind="ExternalInput")
    depB_d = nc.dram_tensor("depB", (128, N), BF16, kind="ExternalInput")
    cwT0_d = nc.dram_tensor("cwT0", (128, C), BF16, kind="ExternalInput")
    cwT1_d = nc.dram_tensor("cwT1", (128, C), BF16, kind="ExternalInput")
    rgbwT_d = nc.dram_tensor("rgbwT", (C, C), BF16, kind="ExternalInput")
    depwT_d = nc.dram_tensor("depwT", (C, C), BF16, kind="ExternalInput")
    bn1_d = nc.dram_tensor("bn1", (C, 2), F32, kind="ExternalInput")
    out_d = nc.dram_tensor("out_vec", (C, 1), F32, kind="ExternalOutput")

    with tile.TileContext(nc) as tc:
        with (
            tc.tile_pool(name="consts", bufs=1) as consts,
            tc.tile_pool(name="big", bufs=1) as big,
            tc.tile_pool(name="jumbo", bufs=2) as jumbo,
            tc.tile_pool(name="small", bufs=4) as small,
            tc.tile_pool(name="ps", bufs=2, space="PSUM") as ps,
        ):
            # ---- consts first (conv needs them), then the big inputs spread
            # over different DMA queues so transfers overlap ----
            cwT0 = consts.tile([128, C], BF16, tag="cwT0")
            cwT1 = consts.tile([128, C], BF16, tag="cwT1")
            rgbwT = consts.tile([C, C], BF16, tag="rgbwT")
            depwT = consts.tile([C, C], BF16, tag="depwT")
            bn1 = consts.tile([C, 2], F32, tag="bn1")
            nc.gpsimd.dma_start(out=cwT0, in_=cwT0_d.ap())
            nc.gpsimd.dma_start(out=cwT1, in_=cwT1_d.ap())
            nc.gpsimd.dma_start(out=rgbwT, in_=rgbwT_d.ap())
            nc.gpsimd.dma_start(out=depwT, in_=depwT_d.ap())
            nc.gpsimd.dma_start(out=bn1, in_=bn1_d.ap())

            # depA/depB land in the jumbo slots later reused by the stored-P
            # tiles (tag sharing => same SBUF space)
            rgb_sb = big.tile([C, N], BF16, tag="rgb")
            depA = jumbo.tile([128, N], BF16, tag="jumbo", padded_shape=[128, 16 * NH])
            depB = jumbo.tile([128, N], BF16, tag="jumbo", padded_shape=[128, 16 * NH])
            nc.sync.dma_start(out=depA, in_=depA_d.ap())
            nc.scalar.dma_start(out=depB, in_=depB_d.ap())
            nc.sync.dma_start(out=rgb_sb, in_=rgb_d.ap())

            # ---- conv1x1(dep) + BN + ReLU -> dep1 (64, 4096) bf16 ----
            dep1 = big.tile([C, N], BF16, tag="dep1")
            rsumd = small.tile([C, 2], F32, tag="rsumd")
            for t in range(4):
                pc = ps.tile([128, 1024], F32, tag="ps")
                for u in range(2):
                    sl = slice(t * 1024 + u * 512, t * 1024 + (u + 1) * 512)
                    nc.tensor.matmul(pc[:C, u * 512:(u + 1) * 512], cwT0, depA[:, sl],
                                     start=True, stop=False)
                    nc.tensor.matmul(pc[:C, u * 512:(u + 1) * 512], cwT1, depB[:, sl],
                                     start=False, stop=True)
                nc.scalar.activation(
                    dep1[:, t * 1024:(t + 1) * 1024], pc[:C, :],
                    mybir.ActivationFunctionType.Relu,
                    bias=bn1[:, 1:2], scale=bn1[:, 0:1],
                    accum_out=rsumd[:, t:t + 1] if t < 2 else None,
                )

            # ---- projections: proj_rgb, proj_dep (64, 4096) bf16 ----
            prgb = big.tile([C, N], BF16, tag="prgb")
            pdep = big.tile([C, N], BF16, tag="pdep")
            for t in range(4):
                pr = ps.tile([128, 1024], F32, tag="ps")
                for u in range(2):
                    sl = slice(t * 1024 + u * 512, t * 1024 + (u + 1) * 512)
                    nc.tensor.matmul(pr[:C, u * 512:(u + 1) * 512], rgbwT, rgb_sb[:, sl],
                                     start=True, stop=True)
                nc.vector.tensor_copy(prgb[:, t * 1024:(t + 1) * 1024], pr[:C, :])
            for t in range(4):
                pd = ps.tile([128, 1024], F32, tag="ps")
                for u in range(2):
                    sl = slice(t * 1024 + u * 512, t * 1024 + (u + 1) * 512)
                    nc.tensor.matmul(pd[:C, u * 512:(u + 1) * 512], depwT, dep1[:, sl],
                                     start=True, stop=True)
                nc.vector.tensor_copy(pdep[:, t * 1024:(t + 1) * 1024], pd[:C, :])

            # ---- prs = proj_rgb + proj_dep; rsum_rgb over own j-half ----
            prs = big.tile([C, N], BF16, tag="prs")
            nc.vector.tensor_add(prs, prgb, pdep)
            rsumr = small.tile([C, 1], F32, tag="rsumr")
            nc.vector.tensor_reduce(rsumr, rgb_sb[:, 0:NH], axis=mybir.AxisListType.X,
                                    op=mybir.AluOpType.add)

            # ---- main attention loop over 16 j-tiles of 128 columns ----
            # P (exp of E^T) fully stored as fp8e5: lo half (i<2048) in the
            # slot vacated by depA, hi half in depB's.
            P_lo = jumbo.tile([128, 16 * NH], FP8, tag="jumbo")
            P_hi = jumbo.tile([128, 16 * NH], FP8, tag="jumbo")
            # 4096/d weights for j-tile pairs, replicated to 64 columns (the
            # s-pass then emits 64-row-replicated chunks = free partition
            # broadcast), fp8e5, DoubleRow-paired along dim 1
            k4096 = consts.tile([128, C], BF16, tag="k4096")
            nc.vector.memset(k4096, 4096.0)
            rdp_tiles = [consts.tile([128, 2, C], FP8, tag=f"rdp{p}", name=f"rdp{p}")
                         for p in range(8)]
            for jt in range(16):
                jsl = slice(jt * 128, (jt + 1) * 128)
                dparts = small.tile([128, 2], F32, tag="dparts")
                for half, P_half in ((0, P_lo), (1, P_hi)):
                    pe = ps.tile([128, 2048], F32, tag="ps")
                    for u in range(4):
                        isl = slice(half * 2048 + u * 512, half * 2048 + (u + 1) * 512)
                        nc.tensor.matmul(pe[:, u * 512:(u + 1) * 512],
                                         pdep[:, jsl], prgb[:, isl],
                                         start=True, stop=True)
                    pview = P_half[:, jt * NH:(jt + 1) * NH]
                    nc.scalar.activation(
                        pview, pe,
                        mybir.ActivationFunctionType.Exp,
                        bias=0.0, scale=0.125,
                        accum_out=dparts[:, half:half + 1],
                    )
                d = small.tile([128, 1], F32, tag="d")
                nc.vector.tensor_reduce(d, dparts, axis=mybir.AxisListType.X,
                                        op=mybir.AluOpType.add)
                rdf = small.tile([128, 1], F32, tag="rdf")
                nc.vector.reciprocal(rdf, d)
                nc.vector.tensor_scalar_mul(rdp_tiles[jt // 2][:, jt % 2, :],
                                            in0=k4096, scalar1=rdf)

            # ---- s-pass: s[i]*4096 = sum_j P[j,i] * (4096/d[j]), DoubleRow
            # fp8 matmuls contracting two j-tiles per pass; i-chunk outer so
            # finished chunks copy out while later chunks accumulate ----
            s_bc = big.tile([C, N], BF16, tag="s_bc")
            P3_lo = P_lo[:, :].rearrange("p (t n) -> p t n", n=NH)
            P3_hi = P_hi[:, :].rearrange("p (t n) -> p t n", n=NH)
            for q in range(8 if stage != "no_s" else 0):
                P3 = P3_lo if q < 4 else P3_hi
                qoff = (q % 4) * 512
                sq = ps.tile([C, 512], F32, tag="ps")
                for pi in range(8):
                    nc.tensor.matmul(sq, rdp_tiles[pi],
                                     P3[:, 2 * pi:2 * pi + 2, qoff:qoff + 512],
                                     start=(pi == 0), stop=(pi == 7),
                                     perf_mode=mybir.MatmulPerfMode.DoubleRow)
                nc.vector.tensor_copy(s_bc[:, q * 512:(q + 1) * 512], sq)

            rtot = small.tile([C, 1], F32, tag="rtot")
            nc.vector.tensor_reduce(rtot, rsumd, axis=mybir.AxisListType.X,
                                    op=mybir.AluOpType.add)
            nc.vector.tensor_add(rtot, rtot, rsumr)
            if stage in ("no_tail", "no_s"):
                nc.sync.dma_start(out=out_d.ap(), in_=rtot)
            else:
                # r12 = sum_i prs[c,i] * s_bc[c,i]/4096; fused multiply +
                # free-dim accumulate (InstTensorScalarPtr; the 1/4096 undoes
                # the fp8-range rescale of rd)
                r12s = small.tile([C, 1], F32, tag="r12s")
                nc.vector.scalar_tensor_tensor(
                    out=dep1[:, :],      # dead scratch by this point
                    in0=prs, scalar=1.0 / 4096.0, in1=s_bc,
                    op0=mybir.AluOpType.mult, op1=mybir.AluOpType.mult,
                    accum_out=r12s,
                )
                outsb = small.tile([C, 1], F32, tag="outsb")
                nc.vector.tensor_add(outsb, r12s, rtot)
                nc.sync.dma_start(out=out_d.ap(), in_=outsb)

    nc.compile()
    nc.finalize()
    return nc


_NC_CACHE = []


def _get_nc():
    if not _NC_CACHE:
        _NC_CACHE.append(build_core_program())
    return _NC_CACHE[0]


def make_in_maps(rgb, dep, conv_w, bn1_g, bn1_b, bn1_m, bn1_v, rgb_w, dep_w):
    s1 = (bn1_g / np.sqrt(bn1_v + EPS)).astype(np.float32).reshape(C, 1)
    b1 = (bn1_b - bn1_m * (bn1_g / np.sqrt(bn1_v + EPS))).astype(np.float32).reshape(C, 1)
    cwT = np.ascontiguousarray(conv_w.T).astype(NPBF16)          # (256, 64)
    rgbwT = np.ascontiguousarray(rgb_w.T).astype(NPBF16)         # (64, 64)
    depwT = np.ascontiguousarray(dep_w.T).astype(NPBF16)
    bn1 = np.ascontiguousarray(np.concatenate([s1, b1], axis=1))
    in_maps = []
    for k in range(8):
        b, h = k // 2, k % 2
        off = h * NH
        r = rgb[b].reshape(C, N)
        d = dep[b].reshape(C1, N)
        r_p = np.concatenate([r[:, off:], r[:, :off]], axis=1).astype(NPBF16)
        d_p = np.concatenate([d[:, off:], d[:, :off]], axis=1).astype(NPBF16)
        in_maps.append({
            "rgb": np.ascontiguousarray(r_p),
            "depA": np.ascontiguousarray(d_p[:128]),
            "depB": np.ascontiguousarray(d_p[128:]),
            "cwT0": np.ascontiguousarray(cwT[:128]),
            "cwT1": np.ascontiguousarray(cwT[128:]),
            "rgbwT": rgbwT,
            "depwT": depwT,
            "bn1": bn1,
        })
    return in_maps


def kernel(rgb, dep, conv_w, bn1_g, bn1_b, bn1_m, bn1_v, rgb_w, dep_w,
           mlp1_w, bn2_g, bn2_b, bn2_m, bn2_v, mlp2_w, bn3_g, bn3_b, bn3_m, bn3_v,
           _trace=False):
    rgb = np.asarray(rgb, dtype=np.float32)
    dep = np.asarray(dep, dtype=np.float32)
    B = rgb.shape[0]
    nc = _get_nc()
    in_maps = make_in_maps(rgb, dep, conv_w, bn1_g, bn1_b, bn1_m, bn1_v, rgb_w, dep_w)
    res = run_bass_kernel_spmd(nc, in_maps, core_ids=list(range(8)), trace=_trace)
    vecs = [res.results[k]["out_vec"].reshape(C) for k in range(8)]
    gap = np.stack([(vecs[2 * b] + vecs[2 * b + 1]) / N for b in range(B)])  # (B, 64)

    # tiny gate MLP on host (part of gather)
    s2 = bn2_g / np.sqrt(bn2_v + EPS)
    bb2 = bn2_b - bn2_m * s2
    s3 = bn3_g / np.sqrt(bn3_v + EPS)
    bb3 = bn3_b - bn3_m * s3
    h = np.maximum(gap @ mlp1_w.T * s2[None, :] + bb2[None, :], 0.0)
    z = h @ mlp2_w.T * s3[None, :] + bb3[None, :]
    gate = 1.0 / (1.0 + np.exp(-z))
    out = gate.reshape(B, C, 1, 1).astype(np.float32)
    if _trace:
        kernel.last_results = res
    return out


# revision 28
# speedup vs baseline: 1.0452x; 1.0452x over previous
"""Trainium2 Bass kernel for the RGB-D cross-attention gate module.

Math shortcut: the module returns only gate = sigmoid(bn3(mlp2(relu(bn2(mlp1(gap))))))
where gap = spatial mean of (att_r + att_b + rgb + dep1).  Summing att_r over
spatial j gives proj_rgb @ s with s[i] = sum_j attn[i, j], so the full N x N
attention never needs materializing - only softmax column denominators d[j]
and the attention row-sum vector s.

Sharding: 8 cores = 4 samples x 2 column-halves of the attention (softmax is
over rows i, so a column shard is fully local; host adds the two 64-vector
partials per sample and applies the tiny MLP during gather).

Per-core pipeline (matmuls bf16, fp32 PSUM accumulate):
  conv1x1+BN+ReLU -> dep1; projections prgb/pdep; E^T tiles (j on partitions,
  i free) -> exp on ACT at (128,2048) grain with accum_out giving d[j] for
  free; P stored in SBUF as fp8e5 (reusing the dep-input slots via tag
  sharing); s-pass contracts P against 4096/d (replicated fp8e5 stationary,
  DoubleRow-paired j-tiles => 0.5 cyc/row) into 64-row-replicated psum chunks
  that double as the partition broadcast; one fused DVE multiply-accumulate
  against prs=(prgb+pdep) with scale 1/4096 plus the rsum terms yields the
  64-vector partial output.
"""

import numpy as np
import ml_dtypes

import concourse.bass as bass
import concourse.bacc as bacc
import concourse.mybir as mybir
import concourse.tile as tile
from concourse.bass_utils import run_bass_kernel_spmd
EPS = 1e-5
N = 4096          # spatial positions (64 x 64)
NH = 2048         # this core's attention-column half
C = 64            # channels
C1 = 256          # dep input channels
BF16 = mybir.dt.bfloat16
FP8 = mybir.dt.float8e5
F32 = mybir.dt.float32
NPBF16 = ml_dtypes.bfloat16


def build_core_program(stage="full"):
    # Bacc (not plain Bass): its compile() pass legalizes multi-sem waits into
    # sequencer instructions; walrus rejects >1 sync wait per compute instr.
    nc = bacc.Bacc("TRN2", target_bir_lowering=False)

    # ---- DRAM I/O (per-core shards; all cores run this same program) ----
    rgb_d = nc.dram_tensor("rgb", (C, N), BF16, kind="ExternalInput")
    depA_d = nc.dram_tensor("depA", (128, N), BF16, kind="ExternalInput")
    depB_d = nc.dram_tensor("depB", (128, N), BF16, kind="ExternalInput")
    cwT0_d = nc.dram_tensor("cwT0", (128, C), BF16, kind="ExternalInput")
    cwT1_d = nc.dram_tensor("cwT1", (128, C), BF16, kind="ExternalInput")
    rgbwT_d = nc.dram_tensor("rgbwT", (C, C), BF16, kind="ExternalInput")
    depwT_d = nc.dram_tensor("depwT", (C, C), BF16, kind="ExternalInput")
    bn1_d = nc.dram_tensor("bn1", (C, 2), F32, kind="ExternalInput")
    out_d = nc.dram_tensor("out_vec", (C, 1), F32, kind="ExternalOutput")

    with tile.TileContext(nc) as tc:
        with (
            tc.tile_pool(name="consts", bufs=1) as consts,
            tc.tile_pool(name="big", bufs=1) as big,
            tc.tile_pool(name="jumbo", bufs=2) as jumbo,
            tc.tile_pool(name="small", bufs=4) as small,
            tc.tile_pool(name="ps", bufs=2, space="PSUM") as ps,
        ):
            # ---- consts first (conv needs them), then the big inputs spread
            # over different DMA queues so transfers overlap ----
            cwT0 = consts.tile([128, C], BF16, tag="cwT0")
            cwT1 = consts.tile([128, C], BF16, tag="cwT1")
            rgbwT = consts.tile([C, C], BF16, tag="rgbwT")
            depwT = consts.tile([C, C], BF16, tag="depwT")
            bn1 = consts.tile([C, 2], F32, tag="bn1")
            nc.gpsimd.dma_start(out=cwT0, in_=cwT0_d.ap())
            nc.gpsimd.dma_start(out=cwT1, in_=cwT1_d.ap())
            nc.gpsimd.dma_start(out=bn1, in_=bn1_d.ap())
            nc.gpsimd.dma_start(out=rgbwT, in_=rgbwT_d.ap())
            nc.gpsimd.dma_start(out=depwT, in_=depwT_d.ap())

            # dep input in per-half chunk tiles, spread over two DMA queues so
            # the conv can start after the first halves land
            rgb_sb = big.tile([C, N], BF16, tag="rgb")
            depA_t = [big.tile([128, NH], BF16, tag=f"depA{t}", name=f"depA{t}")
                      for t in range(2)]
            depB_t = [big.tile([128, NH], BF16, tag=f"depB{t}", name=f"depB{t}")
                      for t in range(2)]
            nc.sync.dma_start(out=depA_t[0], in_=depA_d.ap()[:, 0:NH])
            nc.scalar.dma_start(out=depB_t[0], in_=depB_d.ap()[:, 0:NH])
            nc.sync.dma_start(out=depA_t[1], in_=depA_d.ap()[:, NH:N])
            nc.scalar.dma_start(out=depB_t[1], in_=depB_d.ap()[:, NH:N])
            nc.gpsimd.dma_start(out=rgb_sb, in_=rgb_d.ap())

            # ---- conv1x1(dep) + BN + ReLU -> dep1 (64, 4096) bf16 ----
            dep1 = big.tile([C, N], BF16, tag="dep1")
            rsumd = small.tile([C, 2], F32, tag="rsumd")
            for t in range(4):
                pc = ps.tile([128, 1024], F32, tag="ps")
                for u in range(2):
                    sl = slice((t % 2) * 1024 + u * 512, (t % 2) * 1024 + (u + 1) * 512)
                    nc.tensor.matmul(pc[:C, u * 512:(u + 1) * 512], cwT0,
                                     depA_t[t // 2][:, sl], start=True, stop=False)
                    nc.tensor.matmul(pc[:C, u * 512:(u + 1) * 512], cwT1,
                                     depB_t[t // 2][:, sl], start=False, stop=True)
                nc.scalar.activation(
                    dep1[:, t * 1024:(t + 1) * 1024], pc[:C, :],
                    mybir.ActivationFunctionType.Relu,
                    bias=bn1[:, 1:2], scale=bn1[:, 0:1],
                    accum_out=rsumd[:, t:t + 1] if t < 2 else None,
                )

            # ---- projections: proj_rgb, proj_dep (64, 4096) bf16 ----
            prgb = big.tile([C, N], BF16, tag="prgb")
            pdep = big.tile([C, N], BF16, tag="pdep")
            for t in range(4):
                pr = ps.tile([128, 1024], F32, tag="ps")
                for u in range(2):
                    sl = slice(t * 1024 + u * 512, t * 1024 + (u + 1) * 512)
                    nc.tensor.matmul(pr[:C, u * 512:(u + 1) * 512], rgbwT, rgb_sb[:, sl],
                                     start=True, stop=True)
                nc.vector.tensor_copy(prgb[:, t * 1024:(t + 1) * 1024], pr[:C, :])
            for t in range(4):
                pd = ps.tile([128, 1024], F32, tag="ps")
                for u in range(2):
                    sl = slice(t * 1024 + u * 512, t * 1024 + (u + 1) * 512)
                    nc.tensor.matmul(pd[:C, u * 512:(u + 1) * 512], depwT, dep1[:, sl],
                                     start=True, stop=True)
                nc.vector.tensor_copy(pdep[:, t * 1024:(t + 1) * 1024], pd[:C, :])

            # ---- prs = proj_rgb + proj_dep; rsum_rgb over own j-half ----
            prs = big.tile([C, N], BF16, tag="prs")
            nc.vector.tensor_add(prs, prgb, pdep)
            rsumr = small.tile([C, 1], F32, tag="rsumr")
            nc.vector.tensor_reduce(rsumr, rgb_sb[:, 0:NH], axis=mybir.AxisListType.X,
                                    op=mybir.AluOpType.add)

            # ---- main attention loop over 16 j-tiles of 128 columns ----
            # P (exp of E^T) fully stored as fp8e5: lo half (i<2048) in the
            # slot vacated by depA, hi half in depB's.
            P_lo = jumbo.tile([128, 16 * NH], FP8, tag="jumbo")
            P_hi = jumbo.tile([128, 16 * NH], FP8, tag="jumbo")
            del depA_t, depB_t
            # 4096/d weights for j-tile pairs, replicated to 64 columns (the
            # s-pass then emits 64-row-replicated chunks = free partition
            # broadcast), fp8e5, DoubleRow-paired along dim 1
            k4096 = consts.tile([128, C], BF16, tag="k4096")
            nc.vector.memset(k4096, 4096.0)
            rdp_tiles = [consts.tile([128, 2, C], FP8, tag=f"rdp{p}", name=f"rdp{p}")
                         for p in range(8)]
            for jt in range(16):
                jsl = slice(jt * 128, (jt + 1) * 128)
                dparts = small.tile([128, 2], F32, tag="dparts")
                for half, P_half in ((0, P_lo), (1, P_hi)):
                    pe = ps.tile([128, 2048], F32, tag="ps")
                    for u in range(4):
                        isl = slice(half * 2048 + u * 512, half * 2048 + (u + 1) * 512)
                        nc.tensor.matmul(pe[:, u * 512:(u + 1) * 512],
                                         pdep[:, jsl], prgb[:, isl],
                                         start=True, stop=True)
                    pview = P_half[:, jt * NH:(jt + 1) * NH]
                    nc.scalar.activation(
                        pview, pe,
                        mybir.ActivationFunctionType.Exp,
                        bias=0.0, scale=0.125,
                        accum_out=dparts[:, half:half + 1],
                    )
                d = small.tile([128, 1], F32, tag="d")
                nc.vector.tensor_reduce(d, dparts, axis=mybir.AxisListType.X,
                                        op=mybir.AluOpType.add)
                rdf = small.tile([128, 1], F32, tag="rdf")
                nc.vector.reciprocal(rdf, d)
                nc.vector.tensor_scalar_mul(rdp_tiles[jt // 2][:, jt % 2, :],
                                            in0=k4096, scalar1=rdf)

            # ---- s-pass: s[i]*4096 = sum_j P[j,i] * (4096/d[j]), DoubleRow
            # fp8 matmuls contracting two j-tiles per pass; i-chunk outer so
            # finished chunks copy out while later chunks accumulate ----
            s_bc = big.tile([C, N], BF16, tag="s_bc")
            r12p = small.tile([C, 8], F32, tag="r12p")
            P3_lo = P_lo[:, :].rearrange("p (t n) -> p t n", n=NH)
            P3_hi = P_hi[:, :].rearrange("p (t n) -> p t n", n=NH)
            for q in range(8 if stage != "no_s" else 0):
                P3 = P3_lo if q < 4 else P3_hi
                qoff = (q % 4) * 512
                sq = ps.tile([C, 512], F32, tag="ps")
                for pi in range(8):
                    nc.tensor.matmul(sq, rdp_tiles[pi],
                                     P3[:, 2 * pi:2 * pi + 2, qoff:qoff + 512],
                                     start=(pi == 0), stop=(pi == 7),
                                     perf_mode=mybir.MatmulPerfMode.DoubleRow)
                qsl = slice(q * 512, (q + 1) * 512)
                nc.vector.tensor_copy(s_bc[:, qsl], sq)
                # overlap the r12 chunk-accumulate with later s chunks
                nc.vector.scalar_tensor_tensor(
                    out=dep1[:, qsl], in0=prs[:, qsl], scalar=1.0 / 4096.0,
                    in1=s_bc[:, qsl],
                    op0=mybir.AluOpType.mult, op1=mybir.AluOpType.mult,
                    accum_out=r12p[:, q:q + 1],
                )

            rtot = small.tile([C, 1], F32, tag="rtot")
            nc.vector.tensor_reduce(rtot, rsumd, axis=mybir.AxisListType.X,
                                    op=mybir.AluOpType.add)
            nc.vector.tensor_add(rtot, rtot, rsumr)
            if stage in ("no_tail", "no_s"):
                nc.sync.dma_start(out=out_d.ap(), in_=rtot)
            else:
                r12s = small.tile([C, 1], F32, tag="r12s")
                nc.vector.tensor_reduce(r12s, r12p, axis=mybir.AxisListType.X,
                                        op=mybir.AluOpType.add)
                outsb = small.tile([C, 1], F32, tag="outsb")
                nc.vector.tensor_add(outsb, r12s, rtot)
                nc.sync.dma_start(out=out_d.ap(), in_=outsb)

    nc.compile()
    nc.finalize()
    return nc


_NC_CACHE = []


def _get_nc():
    if not _NC_CACHE:
        _NC_CACHE.append(build_core_program())
    return _NC_CACHE[0]


def make_in_maps(rgb, dep, conv_w, bn1_g, bn1_b, bn1_m, bn1_v, rgb_w, dep_w):
    s1 = (bn1_g / np.sqrt(bn1_v + EPS)).astype(np.float32).reshape(C, 1)
    b1 = (bn1_b - bn1_m * (bn1_g / np.sqrt(bn1_v + EPS))).astype(np.float32).reshape(C, 1)
    cwT = np.ascontiguousarray(conv_w.T).astype(NPBF16)          # (256, 64)
    rgbwT = np.ascontiguousarray(rgb_w.T).astype(NPBF16)         # (64, 64)
    depwT = np.ascontiguousarray(dep_w.T).astype(NPBF16)
    bn1 = np.ascontiguousarray(np.concatenate([s1, b1], axis=1))
    in_maps = []
    for k in range(8):
        b, h = k // 2, k % 2
        off = h * NH
        r = rgb[b].reshape(C, N)
        d = dep[b].reshape(C1, N)
        r_p = np.concatenate([r[:, off:], r[:, :off]], axis=1).astype(NPBF16)
        d_p = np.concatenate([d[:, off:], d[:, :off]], axis=1).astype(NPBF16)
        in_maps.append({
            "rgb": np.ascontiguousarray(r_p),
            "depA": np.ascontiguousarray(d_p[:128]),
            "depB": np.ascontiguousarray(d_p[128:]),
            "cwT0": np.ascontiguousarray(cwT[:128]),
            "cwT1": np.ascontiguousarray(cwT[128:]),
            "rgbwT": rgbwT,
            "depwT": depwT,
            "bn1": bn1,
        })
    return in_maps


def kernel(rgb, dep, conv_w, bn1_g, bn1_b, bn1_m, bn1_v, rgb_w, dep_w,
           mlp1_w, bn2_g, bn2_b, bn2_m, bn2_v, mlp2_w, bn3_g, bn3_b, bn3_m, bn3_v,
           _trace=False):
    rgb = np.asarray(rgb, dtype=np.float32)
    dep = np.asarray(dep, dtype=np.float32)
    B = rgb.shape[0]
    nc = _get_nc()
    in_maps = make_in_maps(rgb, dep, conv_w, bn1_g, bn1_b, bn1_m, bn1_v, rgb_w, dep_w)
    res = run_bass_kernel_spmd(nc, in_maps, core_ids=list(range(8)), trace=_trace)
    vecs = [res.results[k]["out_vec"].reshape(C) for k in range(8)]
    gap = np.stack([(vecs[2 * b] + vecs[2 * b + 1]) / N for b in range(B)])  # (B, 64)

    # tiny gate MLP on host (part of gather)
    s2 = bn2_g / np.sqrt(bn2_v + EPS)
    bb2 = bn2_b - bn2_m * s2
    s3 = bn3_g / np.sqrt(bn3_v + EPS)
    bb3 = bn3_b - bn3_m * s3
    h = np.maximum(gap @ mlp1_w.T * s2[None, :] + bb2[None, :], 0.0)
    z = h @ mlp2_w.T * s3[None, :] + bb3[None, :]
    gate = 1.0 / (1.0 + np.exp(-z))
    out = gate.reshape(B, C, 1, 1).astype(np.float32)
    if _trace:
        kernel.last_results = res
    return out
